# revision 1
# baseline (speedup 1.0000x reference)
"""CIN (nn_CIN_35450660061557) Bass/Tile kernel for 8 TRN2 NeuronCores.

Math (per batch b, embed position d — each (b,d) "column" is independent):
  h_{l+1}[o] = relu( sum_{h,m} Wr_l[o,h,m] * h_l[h] * x0[m] + b_l[o] )
  score[b]   = lb + sum_{l,o,d} lw_l[o] * h_l[o, (b,d)]

Mapping:
  - Data-parallel over batch: 8 cores x 64 batches; N = 64*64 = 4096 columns/core.
  - z_l[(m*H+h), col] = h_l[h,col] * x0[m,col] formed on DVE in bf16 (2x mode),
    one batched tensor_tensor per layer-tile with h broadcast along the m axis;
    layer matmul is PSUM-accumulated over 32 K-chunks of 128 with pre-permuted
    weights WpT[(m,h), o] as stationary operands.
  - z0 = x0 (x) x0 is input-only, precomputed on host, streamed from HBM.
  - x0 column-broadcasts loaded via one 3D HWDGE broadcast DMA per tile.
  - bias+relu on ACT during PSUM->SBUF evacuation.
  - score folds lw into M=1 matmuls; final (d, layer) sum on host (tiny).
"""

import numpy as np
import ml_dtypes

B, M, D = 512, 32, 64
O = 128                      # layer width (all 3 layers)
NCORES = 8
BL = B // NCORES             # 64 batches per core
N = BL * D                   # 4096 columns per core
NT = 512                     # columns per tile
NTILES = N // NT
BF16 = ml_dtypes.bfloat16

_CACHE = {}


def _build():
    from contextlib import ExitStack

    import concourse.bass as bass
    import concourse.mybir as mybir
    import concourse.tile as tile
    from concourse import bacc

    fp32 = mybir.dt.float32
    bf16 = mybir.dt.bfloat16
    Relu = mybir.ActivationFunctionType.Relu

    nc = bacc.Bacc("TRN2", target_bir_lowering=False, debug=False)

    xc_d = nc.dram_tensor("xc", [M, N], bf16, kind="ExternalInput").ap()
    z0_d = nc.dram_tensor("z0", [M * M, N], bf16, kind="ExternalInput").ap()
    w0_d = nc.dram_tensor("w0p", [128, (M * M // 128) * O], bf16, kind="ExternalInput").ap()
    w1_d = nc.dram_tensor("w1p", [128, (O * M // 128) * O], bf16, kind="ExternalInput").ap()
    w2_d = nc.dram_tensor("w2p", [128, (O * M // 128) * O], bf16, kind="ExternalInput").ap()
    b0_d = nc.dram_tensor("b0", [O, 1], fp32, kind="ExternalInput").ap()
    b1_d = nc.dram_tensor("b1", [O, 1], fp32, kind="ExternalInput").ap()
    b2_d = nc.dram_tensor("b2", [O, 1], fp32, kind="ExternalInput").ap()
    lw_d = nc.dram_tensor("lwseg", [O, 3], bf16, kind="ExternalInput").ap()
    out_d = nc.dram_tensor("out", [3, N], fp32, kind="ExternalOutput").ap()

    G0 = (M * M) // 128      # 8 K-chunks for layer 0
    G = (O * M) // 128       # 32 K-chunks for layers 1,2

    with tile.TileContext(nc) as tc, ExitStack() as ctx:
        const = ctx.enter_context(tc.tile_pool(name="const", bufs=1))
        xbp = ctx.enter_context(tc.tile_pool(name="xbp", bufs=2))
        zp = ctx.enter_context(tc.tile_pool(name="zp", bufs=8))
        hp = ctx.enter_context(tc.tile_pool(name="hp", bufs=6))
        sp = ctx.enter_context(tc.tile_pool(name="sp", bufs=3))
        psp = ctx.enter_context(tc.tile_pool(name="psp", bufs=4, space="PSUM"))
        pssp = ctx.enter_context(tc.tile_pool(name="pssp", bufs=3, space="PSUM"))

        def load_z0(nt):
            cs = bass.ts(nt, NT)
            z0v = z0_d[:, cs].rearrange("(g k) c -> k g c", k=128)
            halves = []
            for h in range(2):
                z0t = zp.tile([128, G0 // 2, NT], bf16, name=f"z0t{nt}_{h}", tag="z")
                nc.sync.dma_start(out=z0t, in_=z0v[:, 4 * h : 4 * (h + 1)])
                halves.append(z0t)
            return halves

        def load_xb(nt):
            cs = bass.ts(nt, NT)
            # xb[p, m, c] = x0[m, c] for all p — 3D HWDGE broadcast DMAs
            xb = xbp.tile([128, M, NT], bf16, name=f"xb{nt}", tag="xb")
            for r in range(8):
                nc.sync.dma_start(
                    out=xb[:, 4 * r : 4 * (r + 1)].unsqueeze(1),
                    in_=xc_d[4 * r : 4 * (r + 1), cs]
                    .unsqueeze(0)
                    .partition_broadcast(128),
                )
            return xb

        # ---- constants, interleaved with the first tile's streams so each
        # consumer's first chunk lands just in time ----
        w0s = const.tile([128, G0, O], bf16)
        w1s = const.tile([128, G, O], bf16)
        w2s = const.tile([128, G, O], bf16)
        w1v = w1_d.rearrange("k (g o) -> k g o", o=O)
        w2v = w2_d.rearrange("k (g o) -> k g o", o=O)
        nc.gpsimd.dma_start(out=w0s, in_=w0_d.rearrange("k (g o) -> k g o", o=O))
        nc.gpsimd.dma_start(out=w1s, in_=w1v)
        nc.gpsimd.dma_start(out=w2s, in_=w2v)
        z0_0 = load_z0(0)
        ball = const.tile([O, 3], fp32)
        nc.sync.dma_start(out=ball[:, 0:1], in_=b0_d)
        nc.sync.dma_start(out=ball[:, 1:2], in_=b1_d)
        nc.sync.dma_start(out=ball[:, 2:3], in_=b2_d)
        xb_0 = load_xb(0)
        lws = const.tile([O, 3], bf16)
        nc.sync.dma_start(out=lws, in_=lw_d)
        pre = {0: (z0_0, xb_0)}

        def emit_l0(nt):
            z0halves, _ = pre[nt]
            ps0 = psp.tile([128, NT], fp32, tag="ps", name=f"ps0_{nt}")
            for g in range(G0):
                nc.tensor.matmul(
                    ps0,
                    w0s[:, g],
                    z0halves[g // 4][:, g % 4],
                    start=(g == 0),
                    stop=(g == G0 - 1),
                )
            h1 = hp.tile([128, NT], bf16, tag="h", name=f"h1_{nt}")
            nc.scalar.activation(h1, ps0, Relu, bias=ball[:, 0:1])
            return h1

        def emit_layer(nt, li, hprev, wls):
            xb = pre[nt][1]
            ps = psp.tile([128, NT], fp32, tag="ps", name=f"ps{li + 1}_{nt}")
            hb = hprev.unsqueeze(1)
            # GPSIMD is ~4x slower per element: hand it the final
            # half-quarter first so it overlaps all of the DVE work +
            # the PE's consumption of the earlier chunks.
            zg = zp.tile([128, 6, NT], bf16, tag="z", name=f"zg{li}_{nt}")
            nc.gpsimd.tensor_mul(zg, hb.broadcast_to((128, 6, NT)), xb[:, 26:32])
            for q in range(3):
                zq = zp.tile([128, G0, NT], bf16, tag="z", name=f"zq{li}_{nt}_{q}")
                nc.vector.tensor_mul(
                    zq,
                    hb.broadcast_to((128, G0, NT)),
                    xb[:, G0 * q : G0 * (q + 1)],
                )
                for j in range(G0):
                    k = G0 * q + j
                    nc.tensor.matmul(
                        ps, wls[:, k], zq[:, j], start=(k == 0), stop=False
                    )
            ze = zp.tile([128, 2, NT], bf16, tag="z", name=f"ze{li}_{nt}")
            nc.vector.tensor_mul(ze, hb.broadcast_to((128, 2, NT)), xb[:, 24:26])
            for j in range(2):
                nc.tensor.matmul(ps, wls[:, 24 + j], ze[:, j], start=False, stop=False)
            for j in range(6):
                nc.tensor.matmul(
                    ps, wls[:, 26 + j], zg[:, j], start=False, stop=(j == 5)
                )
            hnext = hp.tile([128, NT], bf16, tag="h", name=f"h{li + 2}_{nt}")
            nc.scalar.activation(hnext, ps, Relu, bias=ball[:, li + 1 : li + 2])
            return hnext

        def emit_score(nt, hs):
            cs = bass.ts(nt, NT)
            for li in range(3):
                pss = pssp.tile([1, NT], fp32, tag="pss")
                nc.tensor.matmul(
                    pss, lws[:, li : li + 1], hs[li], start=True, stop=True
                )
                s_sb = sp.tile([1, NT], fp32, tag="s_sb")
                nc.scalar.copy(s_sb, pss)
                nc.sync.dma_start(out=out_d[li : li + 1, cs], in_=s_sb)

        # Software-pipelined emission: tile nt+1's layer-0 (independent:
        # z0 streams from HBM) is emitted between tile nt's layer-1 and
        # layer-2 blocks, so the PE/DVE relu->z stalls of one tile are
        # filled with the neighbour tile's work.
        h1s = {0: emit_l0(0)}
        for nt in range(NTILES):
            h1 = h1s.pop(nt)
            h2 = emit_layer(nt, 0, h1, w1s)
            if nt + 1 < NTILES:
                pre[nt + 1] = (load_z0(nt + 1), load_xb(nt + 1))
                h1s[nt + 1] = emit_l0(nt + 1)
            h3 = emit_layer(nt, 1, h2, w2s)
            emit_score(nt, [h1, h2, h3])
            del pre[nt]

    nc.compile()
    return nc


def prep_inputs(**inputs):
    """Host-side prep: per-core input maps (shard batch, permute weights)."""
    inp = np.asarray(inputs["input"], np.float32)
    W0 = np.asarray(inputs["W0"], np.float32)
    W1 = np.asarray(inputs["W1"], np.float32)
    W2 = np.asarray(inputs["W2"], np.float32)
    lw = np.asarray(inputs["lw"], np.float32)

    # WpT[(m*H+h), o] = Wr[o, h, m], then SBUF layout [k, (g, o)]:
    # partition k holds row (g*128+k) of WpT for each chunk g.
    def _prep_w(W, H):
        wp = W.reshape(O, H, M).transpose(2, 1, 0).reshape(H * M, O)
        g = H * M // 128
        return np.ascontiguousarray(
            wp.reshape(g, 128, O).transpose(1, 0, 2).reshape(128, g * O)
        ).astype(BF16)

    w0p = _prep_w(W0, M)
    w1p = _prep_w(W1, O)
    w2p = _prep_w(W2, O)
    b0 = np.asarray(inputs["b0"], np.float32).reshape(O, 1)
    b1 = np.asarray(inputs["b1"], np.float32).reshape(O, 1)
    b2 = np.asarray(inputs["b2"], np.float32).reshape(O, 1)
    lwseg = np.ascontiguousarray(lw.reshape(3, O).T).astype(BF16)

    shared = dict(w0p=w0p, w1p=w1p, w2p=w2p, b0=b0, b1=b1, b2=b2, lwseg=lwseg)
    in_maps = []
    for c in range(NCORES):
        xc = np.ascontiguousarray(
            inp[BL * c : BL * (c + 1)].transpose(1, 0, 2).reshape(M, N)
        ).astype(BF16)
        xcf = xc.astype(np.float32)
        z0 = (xcf[:, None, :] * xcf[None, :, :]).reshape(M * M, N).astype(BF16)
        in_maps.append(dict(shared, xc=xc, z0=z0))
    return in_maps


def kernel(**inputs):
    import os

    from concourse import bass_utils

    if "nc" not in _CACHE:
        _CACHE["nc"] = _build()
    nc = _CACHE["nc"]

    in_maps = prep_inputs(**inputs)
    trace = os.environ.get("CIN_TRACE") == "1"
    res = bass_utils.run_bass_kernel_spmd(
        nc, in_maps, core_ids=list(range(NCORES)), trace=trace
    )
    _CACHE["last_res"] = res
    lb = float(np.asarray(inputs["lb"], np.float32).reshape(-1)[0])
    out = np.concatenate(
        [
            res.results[c]["out"].astype(np.float32).sum(0).reshape(BL, D).sum(-1)
            for c in range(NCORES)
        ]
    )
    return out + lb



# revision 5
# speedup vs baseline: 1.7201x; 1.7201x over previous
"""CIN (nn_CIN_35450660061557) Bass/Tile kernel for 8 TRN2 NeuronCores.

Math (per batch b, embed position d — each (b,d) "column" is independent):
  h_{l+1}[o] = relu( sum_{h,m} Wr_l[o,h,m] * h_l[h] * x0[m] + b_l[o] )
  score[b]   = lb + sum_{l,o,d} lw_l[o] * h_l[o, (b,d)]

Mapping (v2 — DMA-descriptor + DVE-mode rework of the v1 kernel):
  - Data-parallel over batch: 8 cores x 64 batches; N = 64*64 = 4096
    columns/core, processed as 4 column-pairs of 1024 (bf16 moving
    operands support N=1024, halving matmul count).
  - Layer 0 uses the symmetry of z0 = x0 (x) x0: folded weights
    W0f[(a,b)] = W0[a,b]+W0[b,a] (a<b) cut K from 1024 to 640 (5
    chunks). z0f is precomputed on host and streamed tile-major so
    each DMA is 128 x 10KB descriptors.
  - x0 column-broadcast: host stores x0 tile-major ([16 blocks of
    (16 m-rows x 512 cols)]) so each broadcast DMA is 128 x 16KB
    contiguous descriptors (v1: 4096 x 1KB lines per tile).
  - h is evacuated from PSUM by the (otherwise idle) ACT engine FOUR
    times into hrep[128, 4, 1024], so every DVE z-multiply
    z[(m,h),c] = h[h,c]*x0[m,c] is a plain strided bf16
    tensor_tensor ([128, 4, 512] per op) eligible for the DVE 2x_1P
    perf mode — v1's stride-0 broadcast APs ran at ~1x.
  - GPSIMD produces 3/16 of each layer's z half-blocks (groups 6B/7A/7B,
    emitted first so they overlap the DVE+PE pipeline).
  - Scores fold lw into M=1 matmuls PSUM-accumulated across the 3
    layers, then a DVE free-axis reduce sums over d on-chip: one
    [1, 64] fp32 DMA per core is the only output traffic.
"""

import numpy as np
import ml_dtypes

B, M, D = 512, 32, 64
O = 128                      # layer width (all 3 layers)
NCORES = 8
BL = B // NCORES             # 64 batches per core
N = BL * D                   # 4096 columns per core
PW = 1024                    # columns per pair (matmul moving width)
NP = N // PW                 # 4 pairs per core
NT = 512                     # columns per xb tile
G0F = 5                      # folded layer-0 K chunks (640 rows)
K0F = G0F * 128
G = 32                       # layer-1/2 K chunks (m index)
BF16 = ml_dtypes.bfloat16

_CACHE = {}


def _fold_pairs():
    """Upper-triangle (a<=b) pair enumeration for the symmetric z0."""
    ia, ib = np.triu_indices(M)
    return ia.astype(np.int64), ib.astype(np.int64)  # 528 pairs


def _build():
    from contextlib import ExitStack

    import concourse.bass as bass
    import concourse.mybir as mybir
    import concourse.tile as tile
    from concourse import bacc

    fp32 = mybir.dt.float32
    bf16 = mybir.dt.bfloat16
    Relu = mybir.ActivationFunctionType.Relu
    Add = mybir.AluOpType.add
    AxX = mybir.AxisListType.X

    nc = bacc.Bacc("TRN2", target_bir_lowering=False, debug=False)

    # xc rows: r = tile*2 + rowhalf -> (16 m-rows x 512 cols) contiguous
    xc_d = nc.dram_tensor("xc", [16, 16 * NT], bf16, kind="ExternalInput").ap()
    # z0 rows: pair*128 + k; per row G0F x 1024 cols contiguous
    z0_d = nc.dram_tensor("z0", [NP * 128, G0F * PW], bf16, kind="ExternalInput").ap()
    w0_d = nc.dram_tensor("w0p", [128, G0F * O], bf16, kind="ExternalInput").ap()
    w1_d = nc.dram_tensor("w1p", [128, G * O], bf16, kind="ExternalInput").ap()
    w2_d = nc.dram_tensor("w2p", [128, G * O], bf16, kind="ExternalInput").ap()
    b0_d = nc.dram_tensor("b0", [O, 1], fp32, kind="ExternalInput").ap()
    b1_d = nc.dram_tensor("b1", [O, 1], fp32, kind="ExternalInput").ap()
    b2_d = nc.dram_tensor("b2", [O, 1], fp32, kind="ExternalInput").ap()
    lw_d = nc.dram_tensor("lwseg", [O, 3], bf16, kind="ExternalInput").ap()
    out_d = nc.dram_tensor("out", [1, BL], fp32, kind="ExternalOutput").ap()

    with tile.TileContext(nc) as tc, ExitStack() as ctx:
        const = ctx.enter_context(tc.tile_pool(name="const", bufs=1))
        xbp = ctx.enter_context(tc.tile_pool(name="xbp", bufs=6))
        z0p = ctx.enter_context(tc.tile_pool(name="z0p", bufs=2))
        zqp = ctx.enter_context(tc.tile_pool(name="zqp", bufs=3))
        zgp = ctx.enter_context(tc.tile_pool(name="zgp", bufs=2))
        hrp = ctx.enter_context(tc.tile_pool(name="hrp", bufs=3))
        h3p = ctx.enter_context(tc.tile_pool(name="h3p", bufs=1))
        psp = ctx.enter_context(tc.tile_pool(name="psp", bufs=3, space="PSUM"))
        pssp = ctx.enter_context(tc.tile_pool(name="pssp", bufs=2, space="PSUM"))

        def load_z0(p):
            z0t = z0p.tile([128, G0F, PW], bf16, name=f"z0t{p}", tag="z0")
            nc.sync.dma_start(
                out=z0t,
                in_=z0_d[bass.ts(p, 128)].rearrange("k (g c) -> k g c", c=PW),
            )
            return z0t

        def load_xb(t, rh):
            # xb[p, ml, c] = x0[rh*16 + ml, t*512 + c] for all 128 p
            xb = xbp.tile([128, 16, NT], bf16, name=f"xb{t}_{rh}", tag="xb")
            nc.sync.dma_start(
                out=xb,
                in_=xc_d[2 * t + rh]
                .unsqueeze(0)
                .rearrange("o (m c) -> o m c", c=NT)
                .partition_broadcast(128),
            )
            return xb

        # ---- constants, interleaved with the first pair's streams ----
        w0s = const.tile([128, G0F, O], bf16)
        w1s = const.tile([128, G, O], bf16)
        w2s = const.tile([128, G, O], bf16)
        nc.gpsimd.dma_start(out=w0s, in_=w0_d.rearrange("k (g o) -> k g o", o=O))
        z0_0 = load_z0(0)
        nc.gpsimd.dma_start(out=w1s, in_=w1_d.rearrange("k (g o) -> k g o", o=O))
        xb_0 = [load_xb(0, 0), load_xb(0, 1), load_xb(1, 0), load_xb(1, 1)]
        nc.gpsimd.dma_start(out=w2s, in_=w2_d.rearrange("k (g o) -> k g o", o=O))
        ball = const.tile([O, 3], fp32)
        nc.sync.dma_start(out=ball[:, 0:1], in_=b0_d)
        nc.sync.dma_start(out=ball[:, 1:2], in_=b1_d)
        nc.sync.dma_start(out=ball[:, 2:3], in_=b2_d)
        lws = const.tile([O, 3], bf16)
        nc.sync.dma_start(out=lws, in_=lw_d)
        out_asm = const.tile([1, BL], fp32)
        pre = {0: (z0_0, xb_0)}

        def evac_hrep(ps, li, p):
            # 4 copies of h so every DVE z-multiply has a unit-stride
            # (non-broadcast) h operand; ACT is otherwise idle.
            hrep = hrp.tile([128, 4, PW], bf16, tag="hr", name=f"hr{li}_{p}")
            for j in range(4):
                for half in range(2):
                    cs = bass.ts(half, NT)
                    nc.scalar.activation(
                        hrep[:, j, cs], ps[half], Relu, bias=ball[:, li : li + 1]
                    )
            return hrep

        def ps_pair(tag, name):
            return [
                psp.tile([128, NT], fp32, tag=f"{tag}{h}", name=f"{name}_{h}")
                for h in range(2)
            ]

        def emit_l0(p):
            z0t = pre[p][0]
            ps0 = ps_pair("ps", f"ps0_{p}")
            for g in range(G0F):
                for half in range(2):
                    cs = bass.ts(half, NT)
                    nc.tensor.matmul(
                        ps0[half],
                        w0s[:, g],
                        z0t[:, g, cs],
                        start=(g == 0),
                        stop=(g == G0F - 1),
                    )
            return evac_hrep(ps0, 0, p)

        def emit_layer(p, li, hrep, wls, last):
            xb = pre[p][1]  # [colhalf*2 + rowhalf]
            ps = ps_pair("ps", f"ps{li + 1}_{p}")

            def zfill(eng, zt, grp, half):
                # z[(m,h),c] for m in [4*grp, 4*grp+4), cols half*512+[0,512)
                cs = bass.ts(half, NT)
                ms = bass.ts(grp % 4, 4)
                eng.tensor_mul(
                    zt[:, :, cs], hrep[:, :, cs], xb[2 * half + grp // 4][:, ms]
                )

            # GPSIMD handles groups 6B, 7A, 7B (slow engine: emit first,
            # consumed last by the PE).
            zg6 = zgp.tile([128, 4, PW], bf16, tag="zg", name=f"zg6_{li}_{p}")
            zg7 = zgp.tile([128, 4, PW], bf16, tag="zg", name=f"zg7_{li}_{p}")
            zfill(nc.gpsimd, zg6, 6, 1)
            zfill(nc.gpsimd, zg7, 7, 0)
            zfill(nc.gpsimd, zg7, 7, 1)

            for grp in range(8):
                if grp < 6:
                    zt = zqp.tile([128, 4, PW], bf16, tag="zq", name=f"zq{grp}_{li}_{p}")
                    zfill(nc.vector, zt, grp, 0)
                    zfill(nc.vector, zt, grp, 1)
                elif grp == 6:
                    zt = zg6
                    zfill(nc.vector, zt, 6, 0)
                else:
                    zt = zg7
                for j in range(4):
                    k = 4 * grp + j
                    for half in range(2):
                        cs = bass.ts(half, NT)
                        nc.tensor.matmul(
                            ps[half],
                            wls[:, k],
                            zt[:, j, cs],
                            start=(k == 0),
                            stop=(k == G - 1),
                        )
            if last:
                h3 = h3p.tile([128, PW], bf16, tag="h3", name=f"h3_{p}")
                for half in range(2):
                    cs = bass.ts(half, NT)
                    nc.scalar.activation(
                        h3[:, cs], ps[half], Relu, bias=ball[:, li + 1 : li + 2]
                    )
                return h3
            return evac_hrep(ps, li + 1, p)

        def emit_score(p, h1, h2, h3):
            for half in range(2):
                cs = bass.ts(half, NT)
                pss = pssp.tile([1, NT], fp32, tag="pss")
                nc.tensor.matmul(pss, lws[:, 0:1], h1[:, 0, cs], start=True, stop=False)
                nc.tensor.matmul(pss, lws[:, 1:2], h2[:, 0, cs], start=False, stop=False)
                nc.tensor.matmul(pss, lws[:, 2:3], h3[:, cs], start=False, stop=True)
                bs = 16 * p + 8 * half
                nc.vector.tensor_reduce(
                    out=out_asm[0:1, bs : bs + 8],
                    in_=pss.rearrange("o (b d) -> o b d", d=D),
                    axis=AxX,
                    op=Add,
                )

        def load_pair(p):
            z0t = load_z0(p)
            xbs = [load_xb(2 * p, 0), load_xb(2 * p, 1), load_xb(2 * p + 1, 0),
                   load_xb(2 * p + 1, 1)]
            return (z0t, xbs)

        # Software-pipelined emission: pair p+1's DMA prefetch and
        # layer-0 are emitted between pair p's layer-1 and layer-2 so
        # the relu->z dependency stalls of one pair are filled with the
        # neighbour pair's independent work.
        h1s = {0: emit_l0(0)}
        for p in range(NP):
            h1 = h1s.pop(p)
            h2 = emit_layer(p, 0, h1, w1s, last=False)
            if p + 1 < NP:
                pre[p + 1] = load_pair(p + 1)
                h1s[p + 1] = emit_l0(p + 1)
            h3 = emit_layer(p, 1, h2, w2s, last=True)
            emit_score(p, h1, h2, h3)
            del pre[p]

        nc.sync.dma_start(out=out_d, in_=out_asm)

    nc.compile()
    return nc


def prep_inputs(**inputs):
    """Host-side prep: per-core input maps (shard batch, permute weights)."""
    inp = np.asarray(inputs["input"], np.float32)
    W0 = np.asarray(inputs["W0"], np.float32)
    W1 = np.asarray(inputs["W1"], np.float32)
    W2 = np.asarray(inputs["W2"], np.float32)
    lw = np.asarray(inputs["lw"], np.float32)

    # Layers 1/2: WpT[(m*H+h), o] = Wr[o, h, m]; SBUF layout [k, (g, o)]
    # with chunk g == m (128 h-rows per chunk).
    def _prep_w(W, H):
        wp = W.reshape(O, H, M).transpose(2, 1, 0).reshape(H * M, O)
        g = H * M // 128
        return np.ascontiguousarray(
            wp.reshape(g, 128, O).transpose(1, 0, 2).reshape(128, g * O)
        ).astype(BF16)

    # Layer 0 folded: K index = upper-tri pair (a<=b) of (h,m); weight
    # W0f[o, (a,b)] = Wr0[o,a,b] + Wr0[o,b,a] (a<b), Wr0[o,a,a] (diag).
    ia, ib = _fold_pairs()
    Wr0 = W0.reshape(O, M, M)
    w0f = Wr0[:, ia, ib] + np.where(ia != ib, 1.0, 0.0)[None, :] * Wr0[:, ib, ia]
    w0f = np.concatenate(
        [w0f, np.zeros((O, K0F - w0f.shape[1]), np.float32)], axis=1
    )  # [O, 640]
    w0p = np.ascontiguousarray(
        w0f.T.reshape(G0F, 128, O).transpose(1, 0, 2).reshape(128, G0F * O)
    ).astype(BF16)

    w1p = _prep_w(W1, O)
    w2p = _prep_w(W2, O)
    b0 = np.asarray(inputs["b0"], np.float32).reshape(O, 1)
    b1 = np.asarray(inputs["b1"], np.float32).reshape(O, 1)
    b2 = np.asarray(inputs["b2"], np.float32).reshape(O, 1)
    lwseg = np.ascontiguousarray(lw.reshape(3, O).T).astype(BF16)

    shared = dict(w0p=w0p, w1p=w1p, w2p=w2p, b0=b0, b1=b1, b2=b2, lwseg=lwseg)
    in_maps = []
    for c in range(NCORES):
        xcore = np.ascontiguousarray(
            inp[BL * c : BL * (c + 1)].transpose(1, 0, 2).reshape(M, N)
        ).astype(BF16)
        # xc tile-major: row r = tile*2 + rowhalf -> 16 m-rows x 512 cols
        xc = np.ascontiguousarray(
            xcore.reshape(2, 16, 8, NT).transpose(2, 0, 1, 3).reshape(16, 16 * NT)
        )
        xf = xcore.astype(np.float32)
        z0f = (xf[ia] * xf[ib]).astype(BF16)  # [528, N]
        z0f = np.concatenate([z0f, np.zeros((K0F - z0f.shape[0], N), BF16)], axis=0)
        # z0 DMA layout: row (pair*128 + k), per row chunks g x 1024 cols
        z0 = np.ascontiguousarray(
            z0f.reshape(G0F, 128, NP, PW).transpose(2, 1, 0, 3).reshape(NP * 128, G0F * PW)
        )
        in_maps.append(dict(shared, xc=xc, z0=z0))
    return in_maps


def kernel(**inputs):
    import os

    from concourse import bass_utils

    if "nc" not in _CACHE:
        _CACHE["nc"] = _build()
    nc = _CACHE["nc"]

    in_maps = prep_inputs(**inputs)
    trace = os.environ.get("CIN_TRACE") == "1"
    res = bass_utils.run_bass_kernel_spmd(
        nc, in_maps, core_ids=list(range(NCORES)), trace=trace
    )
    _CACHE["last_res"] = res
    lb = float(np.asarray(inputs["lb"], np.float32).reshape(-1)[0])
    out = np.concatenate(
        [res.results[c]["out"].astype(np.float32).reshape(BL) for c in range(NCORES)]
    )
    return out + lb


# revision 6
# speedup vs baseline: 2.0650x; 1.2005x over previous
"""CIN (nn_CIN_35450660061557) Bass/Tile kernel for 8 TRN2 NeuronCores.

Math (per batch b, embed position d — each (b,d) "column" is independent):
  h_{l+1}[o] = relu( sum_{h,m} Wr_l[o,h,m] * h_l[h] * x0[m] + b_l[o] )
  score[b]   = lb + sum_{l,o,d} lw_l[o] * h_l[o, (b,d)]

Mapping (v3):
  - Data-parallel over batch: 8 cores x 64 batches; N = 64*64 = 4096
    columns/core, as 4 column-pairs of 1024 = 2 halves of 512.
  - Layer 0 uses the symmetry of z0 = x0 (x) x0: folded weights
    W0f[(a,b)] = W0[a,b]+W0[b,a] (a<b) cut K from 1024 to 640 (5
    chunks); z0f precomputed on host, streamed as 128 x 10KB
    descriptors per pair.
  - x0 column-broadcast: host stores x0 tile-major so each broadcast
    DMA is 128 x 16KB contiguous descriptors.
  - h is evacuated from PSUM by the ACT engine four times per column
    half into hrep[128, 4, 512], so every DVE z-multiply is a plain
    strided bf16 tensor_tensor hitting the 2x_1P perf mode.
  - Half-major matmul order: all 32 half-A matmuls, then half-A evac
    (4 RELUs) overlapping the 32 half-B matmuls — keeps the serial
    RELU chain off the critical path.
  - GPSIMD produces the half-B z of groups 2/5/7 (mid-layer deadlines
    with ~2us slack; emitted at layer start).
  - Scores fold lw into M=1 matmuls PSUM-accumulated across layers;
    DVE reduces over d on-chip: one [1, 64] fp32 DMA per core.
  - Small/constant DMAs ride the ACT HWDGE queue so the first-pair
    bias load is not stuck behind 23us of xb broadcast on SP.
"""

import numpy as np
import ml_dtypes

B, M, D = 512, 32, 64
O = 128                      # layer width (all 3 layers)
NCORES = 8
BL = B // NCORES             # 64 batches per core
N = BL * D                   # 4096 columns per core
PW = 1024                    # columns per pair
NP = N // PW                 # 4 pairs per core
NT = 512                     # columns per half / matmul moving width
G0F = 5                      # folded layer-0 K chunks (640 rows)
K0F = G0F * 128
G = 32                       # layer-1/2 K chunks (m index)
GP_GRPS = (2, 5, 7)          # groups whose half-B z comes from GPSIMD
BF16 = ml_dtypes.bfloat16

_CACHE = {}


def _fold_pairs():
    """Upper-triangle (a<=b) pair enumeration for the symmetric z0."""
    ia, ib = np.triu_indices(M)
    return ia.astype(np.int64), ib.astype(np.int64)  # 528 pairs


def _build():
    from contextlib import ExitStack

    import concourse.bass as bass
    import concourse.mybir as mybir
    import concourse.tile as tile
    from concourse import bacc

    fp32 = mybir.dt.float32
    bf16 = mybir.dt.bfloat16
    Relu = mybir.ActivationFunctionType.Relu
    Add = mybir.AluOpType.add
    AxX = mybir.AxisListType.X

    nc = bacc.Bacc("TRN2", target_bir_lowering=False, debug=False)

    # xc rows: r = tile*2 + rowhalf -> (16 m-rows x 512 cols) contiguous
    xc_d = nc.dram_tensor("xc", [16, 16 * NT], bf16, kind="ExternalInput").ap()
    # z0 rows: pair*128 + k; per row G0F x 1024 cols contiguous
    z0_d = nc.dram_tensor("z0", [NP * 128, G0F * PW], bf16, kind="ExternalInput").ap()
    w0_d = nc.dram_tensor("w0p", [128, G0F * O], bf16, kind="ExternalInput").ap()
    w1_d = nc.dram_tensor("w1p", [128, G * O], bf16, kind="ExternalInput").ap()
    w2_d = nc.dram_tensor("w2p", [128, G * O], bf16, kind="ExternalInput").ap()
    b0_d = nc.dram_tensor("b0", [O, 1], fp32, kind="ExternalInput").ap()
    b1_d = nc.dram_tensor("b1", [O, 1], fp32, kind="ExternalInput").ap()
    b2_d = nc.dram_tensor("b2", [O, 1], fp32, kind="ExternalInput").ap()
    lw_d = nc.dram_tensor("lwseg", [O, 3], bf16, kind="ExternalInput").ap()
    out_d = nc.dram_tensor("out", [1, BL], fp32, kind="ExternalOutput").ap()

    with tile.TileContext(nc) as tc, ExitStack() as ctx:
        const = ctx.enter_context(tc.tile_pool(name="const", bufs=1))
        xbp = ctx.enter_context(tc.tile_pool(name="xbp", bufs=6))
        z0p = ctx.enter_context(tc.tile_pool(name="z0p", bufs=2))
        zqp = ctx.enter_context(tc.tile_pool(name="zqp", bufs=6))
        zgp = ctx.enter_context(tc.tile_pool(name="zgp", bufs=4))
        hrp = ctx.enter_context(tc.tile_pool(name="hrp", bufs=6))
        h3p = ctx.enter_context(tc.tile_pool(name="h3p", bufs=3))
        psp = ctx.enter_context(tc.tile_pool(name="psp", bufs=3, space="PSUM"))
        pssp = ctx.enter_context(tc.tile_pool(name="pssp", bufs=2, space="PSUM"))

        def load_z0(p):
            z0t = z0p.tile([128, G0F, PW], bf16, name=f"z0t{p}", tag="z0")
            nc.sync.dma_start(
                out=z0t,
                in_=z0_d[bass.ts(p, 128)].rearrange("k (g c) -> k g c", c=PW),
            )
            return z0t

        def load_xb(t, rh):
            # xb[p, ml, c] = x0[rh*16 + ml, t*512 + c] for all 128 p
            xb = xbp.tile([128, 16, NT], bf16, name=f"xb{t}_{rh}", tag="xb")
            nc.sync.dma_start(
                out=xb,
                in_=xc_d[2 * t + rh : 2 * t + rh + 1]
                .rearrange("o (m c) -> o m c", c=NT)
                .partition_broadcast(128),
            )
            return xb

        # ---- constants: small loads on the ACT HWDGE queue so they are
        # not serialized behind the first pair's big SP-queue streams ----
        ball = const.tile([O, 3], fp32)
        lws = const.tile([O, 3], bf16)
        nc.scalar.dma_start(out=ball[:, 0:1], in_=b0_d)
        nc.scalar.dma_start(out=ball[:, 1:2], in_=b1_d)
        nc.scalar.dma_start(out=ball[:, 2:3], in_=b2_d)
        nc.scalar.dma_start(out=lws, in_=lw_d)
        w0s = const.tile([128, G0F, O], bf16)
        w1s = const.tile([128, G, O], bf16)
        w2s = const.tile([128, G, O], bf16)
        nc.scalar.dma_start(out=w0s, in_=w0_d.rearrange("k (g o) -> k g o", o=O))
        z0_0 = load_z0(0)
        nc.scalar.dma_start(out=w1s, in_=w1_d.rearrange("k (g o) -> k g o", o=O))
        xb_0 = [load_xb(0, 0), load_xb(0, 1), load_xb(1, 0), load_xb(1, 1)]
        nc.scalar.dma_start(out=w2s, in_=w2_d.rearrange("k (g o) -> k g o", o=O))
        out_asm = const.tile([1, BL], fp32)
        pre = {0: (z0_0, xb_0)}

        def evac_half(ps, li, p, half):
            # 4 copies of this half of h so DVE z-multiplies have
            # unit-stride (non-broadcast) operands; ACT is cheap.
            hr = hrp.tile([128, 4, NT], bf16, tag="hr", name=f"hr{li}_{p}_{half}")
            for j in range(4):
                nc.scalar.activation(hr[:, j], ps, Relu, bias=ball[:, li : li + 1])
            return hr

        def ps_half(tag, name):
            return psp.tile([128, NT], fp32, tag=tag, name=name)

        def emit_l0(p):
            z0t = pre[p][0]
            hrs = []
            for half in range(2):
                cs = bass.ts(half, NT)
                ps0 = ps_half(f"ps{half}", f"ps0_{p}_{half}")
                for g in range(G0F):
                    nc.tensor.matmul(
                        ps0, w0s[:, g], z0t[:, g, cs],
                        start=(g == 0), stop=(g == G0F - 1),
                    )
                hrs.append(evac_half(ps0, 0, p, half))
            return hrs  # [hrA, hrB]

        def emit_layer(p, li, hrA, hrB, wls, last):
            xb = pre[p][1]  # [colhalf*2 + rowhalf]
            hr_in = (hrA, hrB)

            def zfill(eng, zt, grp, half):
                # z[(m,h),c] for m in [4*grp, 4*grp+4), cols half*512+[0,512)
                ms = bass.ts(grp % 4, 4)
                eng.tensor_mul(zt, hr_in[half], xb[2 * half + grp // 4][:, ms])

            # GPSIMD half-B z for groups 2/5/7 (mid-layer deadlines).
            zgB = {}
            for grp in GP_GRPS:
                zgB[grp] = zgp.tile(
                    [128, 4, NT], bf16, tag="zg", name=f"zg{grp}_{li}_{p}"
                )
                zfill(nc.gpsimd, zgB[grp], grp, 1)

            outs = []
            for half in range(2):
                ps = ps_half(f"ps{half}", f"ps{li + 1}_{p}_{half}")
                for grp in range(8):
                    if half == 1 and grp in zgB:
                        zt = zgB[grp]
                    else:
                        zt = zqp.tile(
                            [128, 4, NT], bf16, tag="zq",
                            name=f"zq{grp}_{li}_{p}_{half}",
                        )
                        zfill(nc.vector, zt, grp, half)
                    for j in range(4):
                        k = 4 * grp + j
                        nc.tensor.matmul(
                            ps, wls[:, k], zt[:, j],
                            start=(k == 0), stop=(k == G - 1),
                        )
                if last:
                    h3 = h3p.tile([128, NT], bf16, tag="h3", name=f"h3_{p}_{half}")
                    nc.scalar.activation(h3, ps, Relu, bias=ball[:, li + 1 : li + 2])
                    outs.append(h3)
                else:
                    outs.append(evac_half(ps, li + 1, p, half))
            return outs

        def emit_score(p, hs1, hs2, hs3):
            for half in range(2):
                pss = pssp.tile([1, NT], fp32, tag="pss")
                nc.tensor.matmul(
                    pss, lws[:, 0:1], hs1[half][:, 0], start=True, stop=False
                )
                nc.tensor.matmul(
                    pss, lws[:, 1:2], hs2[half][:, 0], start=False, stop=False
                )
                nc.tensor.matmul(
                    pss, lws[:, 2:3], hs3[half], start=False, stop=True
                )
                bs = 16 * p + 8 * half
                nc.vector.tensor_reduce(
                    out=out_asm[0:1, bs : bs + 8],
                    in_=pss.rearrange("o (b d) -> o b d", d=D),
                    axis=AxX,
                    op=Add,
                )

        def load_pair(p):
            z0t = load_z0(p)
            xbs = [load_xb(2 * p, 0), load_xb(2 * p, 1), load_xb(2 * p + 1, 0),
                   load_xb(2 * p + 1, 1)]
            return (z0t, xbs)

        # Software-pipelined emission: pair p+1's DMA prefetch and
        # layer-0 are emitted between pair p's layer-1 and layer-2.
        h1s = {0: emit_l0(0)}
        for p in range(NP):
            hs1 = h1s.pop(p)
            hs2 = emit_layer(p, 0, hs1[0], hs1[1], w1s, last=False)
            if p + 1 < NP:
                pre[p + 1] = load_pair(p + 1)
                h1s[p + 1] = emit_l0(p + 1)
            hs3 = emit_layer(p, 1, hs2[0], hs2[1], w2s, last=True)
            emit_score(p, hs1, hs2, hs3)
            del pre[p]

        nc.scalar.dma_start(out=out_d, in_=out_asm)

    nc.compile()
    return nc


def prep_inputs(**inputs):
    """Host-side prep: per-core input maps (shard batch, permute weights)."""
    inp = np.asarray(inputs["input"], np.float32)
    W0 = np.asarray(inputs["W0"], np.float32)
    W1 = np.asarray(inputs["W1"], np.float32)
    W2 = np.asarray(inputs["W2"], np.float32)
    lw = np.asarray(inputs["lw"], np.float32)

    # Layers 1/2: WpT[(m*H+h), o] = Wr[o, h, m]; SBUF layout [k, (g, o)]
    # with chunk g == m (128 h-rows per chunk).
    def _prep_w(W, H):
        wp = W.reshape(O, H, M).transpose(2, 1, 0).reshape(H * M, O)
        g = H * M // 128
        return np.ascontiguousarray(
            wp.reshape(g, 128, O).transpose(1, 0, 2).reshape(128, g * O)
        ).astype(BF16)

    # Layer 0 folded: K index = upper-tri pair (a<=b); weight
    # W0f[o, (a,b)] = Wr0[o,a,b] + Wr0[o,b,a] (a<b), Wr0[o,a,a] (diag).
    ia, ib = _fold_pairs()
    Wr0 = W0.reshape(O, M, M)
    w0f = Wr0[:, ia, ib] + np.where(ia != ib, 1.0, 0.0)[None, :] * Wr0[:, ib, ia]
    w0f = np.concatenate(
        [w0f, np.zeros((O, K0F - w0f.shape[1]), np.float32)], axis=1
    )  # [O, 640]
    w0p = np.ascontiguousarray(
        w0f.T.reshape(G0F, 128, O).transpose(1, 0, 2).reshape(128, G0F * O)
    ).astype(BF16)

    w1p = _prep_w(W1, O)
    w2p = _prep_w(W2, O)
    b0 = np.asarray(inputs["b0"], np.float32).reshape(O, 1)
    b1 = np.asarray(inputs["b1"], np.float32).reshape(O, 1)
    b2 = np.asarray(inputs["b2"], np.float32).reshape(O, 1)
    lwseg = np.ascontiguousarray(lw.reshape(3, O).T).astype(BF16)

    shared = dict(w0p=w0p, w1p=w1p, w2p=w2p, b0=b0, b1=b1, b2=b2, lwseg=lwseg)
    in_maps = []
    for c in range(NCORES):
        xcore = np.ascontiguousarray(
            inp[BL * c : BL * (c + 1)].transpose(1, 0, 2).reshape(M, N)
        ).astype(BF16)
        # xc tile-major: row r = tile*2 + rowhalf -> 16 m-rows x 512 cols
        xc = np.ascontiguousarray(
            xcore.reshape(2, 16, 8, NT).transpose(2, 0, 1, 3).reshape(16, 16 * NT)
        )
        xf = xcore.astype(np.float32)
        z0f = (xf[ia] * xf[ib]).astype(BF16)  # [528, N]
        z0f = np.concatenate([z0f, np.zeros((K0F - z0f.shape[0], N), BF16)], axis=0)
        # z0 DMA layout: row (pair*128 + k), per row chunks g x 1024 cols
        z0 = np.ascontiguousarray(
            z0f.reshape(G0F, 128, NP, PW).transpose(2, 1, 0, 3).reshape(NP * 128, G0F * PW)
        )
        in_maps.append(dict(shared, xc=xc, z0=z0))
    return in_maps


def kernel(**inputs):
    import os

    from concourse import bass_utils

    if "nc" not in _CACHE:
        _CACHE["nc"] = _build()
    nc = _CACHE["nc"]

    in_maps = prep_inputs(**inputs)
    trace = os.environ.get("CIN_TRACE") == "1"
    res = bass_utils.run_bass_kernel_spmd(
        nc, in_maps, core_ids=list(range(NCORES)), trace=trace
    )
    _CACHE["last_res"] = res
    lb = float(np.asarray(inputs["lb"], np.float32).reshape(-1)[0])
    out = np.concatenate(
        [res.results[c]["out"].astype(np.float32).reshape(BL) for c in range(NCORES)]
    )
    return out + lb


# revision 11
# speedup vs baseline: 2.0987x; 1.0163x over previous
"""CIN (nn_CIN_35450660061557) Bass/Tile kernel for 8 TRN2 NeuronCores.

Math (per batch b, embed position d — each (b,d) "column" is independent):
  h_{l+1}[o] = relu( sum_{h,m} Wr_l[o,h,m] * h_l[h] * x0[m] + b_l[o] )
  score[b]   = lb + sum_{l,o,d} lw_l[o] * h_l[o, (b,d)]

Mapping (v3):
  - Data-parallel over batch: 8 cores x 64 batches; N = 64*64 = 4096
    columns/core, as 4 column-pairs of 1024 = 2 halves of 512.
  - Layer 0 uses the symmetry of z0 = x0 (x) x0: folded weights
    W0f[(a,b)] = W0[a,b]+W0[b,a] (a<b) cut K from 1024 to 640 (5
    chunks); z0f precomputed on host, streamed as 128 x 10KB
    descriptors per pair.
  - x0 column-broadcast: host stores x0 tile-major so each broadcast
    DMA is 128 x 16KB contiguous descriptors.
  - h is evacuated from PSUM by the ACT engine four times per column
    half into hrep[128, 4, 512], so every DVE z-multiply is a plain
    strided bf16 tensor_tensor hitting the 2x_1P perf mode.
  - Half-major matmul order: all 32 half-A matmuls, then half-A evac
    (4 RELUs) overlapping the 32 half-B matmuls — keeps the serial
    RELU chain off the critical path.
  - GPSIMD produces the half-B z of groups 2/5/7 (mid-layer deadlines
    with ~2us slack; emitted at layer start).
  - Scores fold lw into M=1 matmuls PSUM-accumulated across layers;
    DVE reduces over d on-chip: one [1, 64] fp32 DMA per core.
  - Small/constant DMAs ride the ACT HWDGE queue so the first-pair
    bias load is not stuck behind 23us of xb broadcast on SP.
"""

import numpy as np
import ml_dtypes

B, M, D = 512, 32, 64
O = 128                      # layer width (all 3 layers)
NCORES = 8
BL = B // NCORES             # 64 batches per core
N = BL * D                   # 4096 columns per core
PW = 1024                    # columns per pair
NP = N // PW                 # 4 pairs per core
NT = 512                     # columns per half / matmul moving width
G0F = 5                      # folded layer-0 K chunks (640 rows)
K0F = G0F * 128
G = 32                       # layer-1/2 K chunks (m index)
GP_GRPS = (2, 5, 7)          # groups whose half-B z comes from GPSIMD
BF16 = ml_dtypes.bfloat16

_CACHE = {}


def _fold_pairs():
    """Upper-triangle (a<=b) pair enumeration for the symmetric z0."""
    ia, ib = np.triu_indices(M)
    return ia.astype(np.int64), ib.astype(np.int64)  # 528 pairs


def _build():
    from contextlib import ExitStack

    import concourse.bass as bass
    import concourse.mybir as mybir
    import concourse.tile as tile
    from concourse import bacc

    fp32 = mybir.dt.float32
    bf16 = mybir.dt.bfloat16
    Relu = mybir.ActivationFunctionType.Relu
    Add = mybir.AluOpType.add
    AxX = mybir.AxisListType.X

    nc = bacc.Bacc("TRN2", target_bir_lowering=False, debug=False)

    # xc rows: r = tile*2 + rowhalf -> (16 m-rows x 512 cols) contiguous
    xc_d = nc.dram_tensor("xc", [16, 16 * NT], bf16, kind="ExternalInput").ap()
    # z0 rows: pair*128 + k; per row G0F x 1024 cols contiguous
    z0_d = nc.dram_tensor("z0", [NP * 128, G0F * PW], bf16, kind="ExternalInput").ap()
    w0_d = nc.dram_tensor("w0p", [128, G0F * O], bf16, kind="ExternalInput").ap()
    w1_d = nc.dram_tensor("w1p", [128, G * O], bf16, kind="ExternalInput").ap()
    w2_d = nc.dram_tensor("w2p", [128, G * O], bf16, kind="ExternalInput").ap()
    b0_d = nc.dram_tensor("b0", [O, 1], fp32, kind="ExternalInput").ap()
    b1_d = nc.dram_tensor("b1", [O, 1], fp32, kind="ExternalInput").ap()
    b2_d = nc.dram_tensor("b2", [O, 1], fp32, kind="ExternalInput").ap()
    lw_d = nc.dram_tensor("lwseg", [O, 3], bf16, kind="ExternalInput").ap()
    out_d = nc.dram_tensor("out", [1, BL], fp32, kind="ExternalOutput").ap()

    with tile.TileContext(nc) as tc, ExitStack() as ctx:
        const = ctx.enter_context(tc.tile_pool(name="const", bufs=1))
        xbp = ctx.enter_context(tc.tile_pool(name="xbp", bufs=5))
        z0p = ctx.enter_context(tc.tile_pool(name="z0p", bufs=2))
        zqp = ctx.enter_context(tc.tile_pool(name="zqp", bufs=10))
        zgp = ctx.enter_context(tc.tile_pool(name="zgp", bufs=4))
        hrp = ctx.enter_context(tc.tile_pool(name="hrp", bufs=6))
        h3p = ctx.enter_context(tc.tile_pool(name="h3p", bufs=4))
        psp = ctx.enter_context(tc.tile_pool(name="psp", bufs=3, space="PSUM"))
        pssp = ctx.enter_context(tc.tile_pool(name="pssp", bufs=2, space="PSUM"))

        def load_z0(p):
            # z0 rides the ACT HWDGE queue: after startup that queue is
            # empty, so layer-0's stream never waits behind xb broadcasts
            # (and neither does the batched DMA-completion semaphore the
            # first layer-0 matmul waits on).
            z0t = z0p.tile([128, G0F, PW], bf16, name=f"z0t{p}", tag="z0")
            nc.scalar.dma_start(
                out=z0t,
                in_=z0_d[bass.ts(p, 128)].rearrange("k (g c) -> k g c", c=PW),
            )
            return z0t

        def load_xb(t, rh):
            # xb[p, ml, c] = x0[rh*16 + ml, t*512 + c] for all 128 p
            xb = xbp.tile([128, 16, NT], bf16, name=f"xb{t}_{rh}", tag="xb")
            nc.sync.dma_start(
                out=xb,
                in_=xc_d[2 * t + rh : 2 * t + rh + 1]
                .rearrange("o (m c) -> o m c", c=NT)
                .partition_broadcast(128),
            )
            return xb

        # ---- constants: small loads on the ACT HWDGE queue so they are
        # not serialized behind the first pair's big SP-queue streams ----
        ball = const.tile([O, 3], fp32)
        lws = const.tile([O, 3], bf16)
        nc.scalar.dma_start(out=ball[:, 0:1], in_=b0_d)
        nc.scalar.dma_start(out=ball[:, 1:2], in_=b1_d)
        nc.scalar.dma_start(out=ball[:, 2:3], in_=b2_d)
        nc.scalar.dma_start(out=lws, in_=lw_d)
        w0s = const.tile([128, G0F, O], bf16)
        w1s = const.tile([128, G, O], bf16)
        w2s = const.tile([128, G, O], bf16)
        nc.scalar.dma_start(out=w0s, in_=w0_d.rearrange("k (g o) -> k g o", o=O))
        z0_0 = load_z0(0)
        xb_0 = [load_xb(0, 0), load_xb(0, 1), load_xb(1, 0), load_xb(1, 1)]
        nc.scalar.dma_start(out=w1s, in_=w1_d.rearrange("k (g o) -> k g o", o=O))
        nc.scalar.dma_start(out=w2s, in_=w2_d.rearrange("k (g o) -> k g o", o=O))
        out_asm = const.tile([1, BL], fp32)
        pre = {0: (z0_0, xb_0)}

        def evac_half(ps, li, p, half):
            # 4 copies of this half of h so DVE z-multiplies have
            # unit-stride (non-broadcast) operands; ACT is cheap.
            hr = hrp.tile([128, 4, NT], bf16, tag="hr", name=f"hr{li}_{p}_{half}")
            for j in range(4):
                nc.scalar.activation(hr[:, j], ps, Relu, bias=ball[:, li : li + 1])
            return hr

        def ps_half(tag, name):
            return psp.tile([128, NT], fp32, tag=tag, name=name)

        def emit_l0(p):
            z0t = pre[p][0]
            hrs = []
            for half in range(2):
                cs = bass.ts(half, NT)
                ps0 = ps_half(f"ps{half}", f"ps0_{p}_{half}")
                for g in range(G0F):
                    nc.tensor.matmul(
                        ps0, w0s[:, g], z0t[:, g, cs],
                        start=(g == 0), stop=(g == G0F - 1),
                    )
                hrs.append(evac_half(ps0, 0, p, half))
            return hrs  # [hrA, hrB]

        def emit_layer(p, li, hrA, hrB, wls, last):
            xb = pre[p][1]  # [colhalf*2 + rowhalf]
            hr_in = (hrA, hrB)

            def zfill(eng, zt, grp, half):
                # z[(m,h),c] for m in [4*grp, 4*grp+4), cols half*512+[0,512)
                ms = bass.ts(grp % 4, 4)
                eng.tensor_mul(zt, hr_in[half], xb[2 * half + grp // 4][:, ms])

            # GPSIMD half-B z for groups 2/5/7 (mid-layer deadlines).
            zgB = {}
            for grp in GP_GRPS:
                zgB[grp] = zgp.tile(
                    [128, 4, NT], bf16, tag="zg", name=f"zg{grp}_{li}_{p}"
                )
                zfill(nc.gpsimd, zgB[grp], grp, 1)

            outs = []
            for half in range(2):
                ps = ps_half(f"ps{half}", f"ps{li + 1}_{p}_{half}")
                for grp in range(8):
                    if half == 1 and grp in zgB:
                        zt = zgB[grp]
                    else:
                        zt = zqp.tile(
                            [128, 4, NT], bf16, tag="zq",
                            name=f"zq{grp}_{li}_{p}_{half}",
                        )
                        zfill(nc.vector, zt, grp, half)
                    for j in range(4):
                        k = 4 * grp + j
                        nc.tensor.matmul(
                            ps, wls[:, k], zt[:, j],
                            start=(k == 0), stop=(k == G - 1),
                        )
                if last:
                    h3 = h3p.tile([128, NT], bf16, tag="h3", name=f"h3_{p}_{half}")
                    nc.scalar.activation(h3, ps, Relu, bias=ball[:, li + 1 : li + 2])
                    outs.append(h3)
                else:
                    outs.append(evac_half(ps, li + 1, p, half))
            return outs

        def emit_score(p, hs1, hs2, hs3):
            for half in range(2):
                pss = pssp.tile([1, NT], fp32, tag="pss")
                nc.tensor.matmul(
                    pss, lws[:, 0:1], hs1[half][:, 0], start=True, stop=False
                )
                nc.tensor.matmul(
                    pss, lws[:, 1:2], hs2[half][:, 0], start=False, stop=False
                )
                nc.tensor.matmul(
                    pss, lws[:, 2:3], hs3[half], start=False, stop=True
                )
                bs = 16 * p + 8 * half
                nc.vector.tensor_reduce(
                    out=out_asm[0:1, bs : bs + 8],
                    in_=pss.rearrange("o (b d) -> o b d", d=D),
                    axis=AxX,
                    op=Add,
                )

        def load_pair(p):
            z0t = load_z0(p)
            xbs = [load_xb(2 * p, 0), load_xb(2 * p, 1), load_xb(2 * p + 1, 0),
                   load_xb(2 * p + 1, 1)]
            return (z0t, xbs)

        # Software-pipelined emission: pair p+1's DMA prefetch and
        # layer-0 are emitted between pair p's layer-1 and layer-2, and
        # pair p's scores are emitted one iteration LATE (mid pair p+1)
        # so the score matmuls + DVE reduces never head-of-line-block
        # the next pair's z fills on the in-order engine queues.
        h1s = {0: emit_l0(0)}
        scoreq = {}
        for p in range(NP):
            hs1 = h1s.pop(p)
            if p - 1 in scoreq:
                emit_score(p - 1, *scoreq.pop(p - 1))
            hs2 = emit_layer(p, 0, hs1[0], hs1[1], w1s, last=False)
            if p + 1 < NP:
                pre[p + 1] = load_pair(p + 1)
                h1s[p + 1] = emit_l0(p + 1)
            hs3 = emit_layer(p, 1, hs2[0], hs2[1], w2s, last=True)
            scoreq[p] = (hs1, hs2, hs3)
            del pre[p]
        emit_score(NP - 1, *scoreq.pop(NP - 1))

        nc.scalar.dma_start(out=out_d, in_=out_asm)

    nc.compile()
    return nc


def prep_inputs(**inputs):
    """Host-side prep: per-core input maps (shard batch, permute weights)."""
    inp = np.asarray(inputs["input"], np.float32)
    W0 = np.asarray(inputs["W0"], np.float32)
    W1 = np.asarray(inputs["W1"], np.float32)
    W2 = np.asarray(inputs["W2"], np.float32)
    lw = np.asarray(inputs["lw"], np.float32)

    # Layers 1/2: WpT[(m*H+h), o] = Wr[o, h, m]; SBUF layout [k, (g, o)]
    # with chunk g == m (128 h-rows per chunk).
    def _prep_w(W, H):
        wp = W.reshape(O, H, M).transpose(2, 1, 0).reshape(H * M, O)
        g = H * M // 128
        return np.ascontiguousarray(
            wp.reshape(g, 128, O).transpose(1, 0, 2).reshape(128, g * O)
        ).astype(BF16)

    # Layer 0 folded: K index = upper-tri pair (a<=b); weight
    # W0f[o, (a,b)] = Wr0[o,a,b] + Wr0[o,b,a] (a<b), Wr0[o,a,a] (diag).
    ia, ib = _fold_pairs()
    Wr0 = W0.reshape(O, M, M)
    w0f = Wr0[:, ia, ib] + np.where(ia != ib, 1.0, 0.0)[None, :] * Wr0[:, ib, ia]
    w0f = np.concatenate(
        [w0f, np.zeros((O, K0F - w0f.shape[1]), np.float32)], axis=1
    )  # [O, 640]
    w0p = np.ascontiguousarray(
        w0f.T.reshape(G0F, 128, O).transpose(1, 0, 2).reshape(128, G0F * O)
    ).astype(BF16)

    w1p = _prep_w(W1, O)
    w2p = _prep_w(W2, O)
    b0 = np.asarray(inputs["b0"], np.float32).reshape(O, 1)
    b1 = np.asarray(inputs["b1"], np.float32).reshape(O, 1)
    b2 = np.asarray(inputs["b2"], np.float32).reshape(O, 1)
    lwseg = np.ascontiguousarray(lw.reshape(3, O).T).astype(BF16)

    shared = dict(w0p=w0p, w1p=w1p, w2p=w2p, b0=b0, b1=b1, b2=b2, lwseg=lwseg)
    in_maps = []
    for c in range(NCORES):
        xcore = np.ascontiguousarray(
            inp[BL * c : BL * (c + 1)].transpose(1, 0, 2).reshape(M, N)
        ).astype(BF16)
        # xc tile-major: row r = tile*2 + rowhalf -> 16 m-rows x 512 cols
        xc = np.ascontiguousarray(
            xcore.reshape(2, 16, 8, NT).transpose(2, 0, 1, 3).reshape(16, 16 * NT)
        )
        xf = xcore.astype(np.float32)
        z0f = (xf[ia] * xf[ib]).astype(BF16)  # [528, N]
        z0f = np.concatenate([z0f, np.zeros((K0F - z0f.shape[0], N), BF16)], axis=0)
        # z0 DMA layout: row (pair*128 + k), per row chunks g x 1024 cols
        z0 = np.ascontiguousarray(
            z0f.reshape(G0F, 128, NP, PW).transpose(2, 1, 0, 3).reshape(NP * 128, G0F * PW)
        )
        in_maps.append(dict(shared, xc=xc, z0=z0))
    return in_maps


def kernel(**inputs):
    import os

    from concourse import bass_utils

    if "nc" not in _CACHE:
        _CACHE["nc"] = _build()
    nc = _CACHE["nc"]

    in_maps = prep_inputs(**inputs)
    trace = os.environ.get("CIN_TRACE") == "1"
    res = bass_utils.run_bass_kernel_spmd(
        nc, in_maps, core_ids=list(range(NCORES)), trace=trace
    )
    _CACHE["last_res"] = res
    lb = float(np.asarray(inputs["lb"], np.float32).reshape(-1)[0])
    out = np.concatenate(
        [res.results[c]["out"].astype(np.float32).reshape(BL) for c in range(NCORES)]
    )
    return out + lb


# revision 14
# speedup vs baseline: 2.2657x; 1.0796x over previous
"""CIN (nn_CIN_35450660061557) Bass/Tile kernel for 8 TRN2 NeuronCores.

Math (per batch b, embed position d — each (b,d) "column" is independent):
  h_{l+1}[o] = relu( sum_{h,m} Wr_l[o,h,m] * h_l[h] * x0[m] + b_l[o] )
  score[b]   = lb + sum_{l,o,d} lw_l[o] * h_l[o, (b,d)]

Mapping (v3):
  - Data-parallel over batch: 8 cores x 64 batches; N = 64*64 = 4096
    columns/core, as 4 column-pairs of 1024 = 2 halves of 512.
  - Layer 0 uses the symmetry of z0 = x0 (x) x0: folded weights
    W0f[(a,b)] = W0[a,b]+W0[b,a] (a<b) cut K from 1024 to 640 (5
    chunks); z0f precomputed on host, streamed as 128 x 10KB
    descriptors per pair.
  - x0 column-broadcast: host stores x0 tile-major so each broadcast
    DMA is 128 x 16KB contiguous descriptors.
  - h is evacuated from PSUM by the ACT engine four times per column
    half into hrep[128, 4, 512], so every DVE z-multiply is a plain
    strided bf16 tensor_tensor hitting the 2x_1P perf mode.
  - Half-major matmul order: all 32 half-A matmuls, then half-A evac
    (4 RELUs) overlapping the 32 half-B matmuls — keeps the serial
    RELU chain off the critical path.
  - GPSIMD produces the half-B z of groups 2/5/7 (mid-layer deadlines
    with ~2us slack; emitted at layer start).
  - Scores fold lw into M=1 matmuls PSUM-accumulated across layers;
    DVE reduces over d on-chip: one [1, 64] fp32 DMA per core.
  - Small/constant DMAs ride the ACT HWDGE queue so the first-pair
    bias load is not stuck behind 23us of xb broadcast on SP.
"""

import numpy as np
import ml_dtypes

B, M, D = 512, 32, 64
O = 128                      # layer width (all 3 layers)
NCORES = 8
BL = B // NCORES             # 64 batches per core
N = BL * D                   # 4096 columns per core
PW = 1024                    # columns per pair
NP = N // PW                 # 4 pairs per core
NT = 512                     # columns per half / matmul moving width
G0F = 5                      # folded layer-0 K chunks (640 rows)
K0F = G0F * 128
G = 32                       # layer-1/2 K chunks (m index)
GP_GRPS = (3, 6)             # groups whose half-B z comes from GPSIMD
BF16 = ml_dtypes.bfloat16

_CACHE = {}


def _fold_pairs():
    """Upper-triangle (a<=b) pair enumeration for the symmetric z0."""
    ia, ib = np.triu_indices(M)
    return ia.astype(np.int64), ib.astype(np.int64)  # 528 pairs


def _build():
    from contextlib import ExitStack

    import concourse.bass as bass
    import concourse.mybir as mybir
    import concourse.tile as tile
    from concourse import bacc

    fp32 = mybir.dt.float32
    bf16 = mybir.dt.bfloat16
    Relu = mybir.ActivationFunctionType.Relu
    Add = mybir.AluOpType.add
    AxX = mybir.AxisListType.X

    nc = bacc.Bacc("TRN2", target_bir_lowering=False, debug=False)

    # xc rows: r = tile*2 + rowhalf -> (16 m-rows x 512 cols) contiguous
    xc_d = nc.dram_tensor("xc", [16, 16 * NT], bf16, kind="ExternalInput").ap()
    # z0 rows: pair*128 + k; per row G0F x 1024 cols contiguous
    z0_d = nc.dram_tensor("z0", [NP * 128, G0F * PW], bf16, kind="ExternalInput").ap()
    w0_d = nc.dram_tensor("w0p", [128, G0F * O], bf16, kind="ExternalInput").ap()
    w1_d = nc.dram_tensor("w1p", [128, G * O], bf16, kind="ExternalInput").ap()
    w2_d = nc.dram_tensor("w2p", [128, G * O], bf16, kind="ExternalInput").ap()
    b0_d = nc.dram_tensor("b0", [O, 1], fp32, kind="ExternalInput").ap()
    b1_d = nc.dram_tensor("b1", [O, 1], fp32, kind="ExternalInput").ap()
    b2_d = nc.dram_tensor("b2", [O, 1], fp32, kind="ExternalInput").ap()
    lw_d = nc.dram_tensor("lwseg", [O, 3], bf16, kind="ExternalInput").ap()
    out_d = nc.dram_tensor("out", [1, BL], fp32, kind="ExternalOutput").ap()

    with tile.TileContext(nc) as tc, ExitStack() as ctx:
        const = ctx.enter_context(tc.tile_pool(name="const", bufs=1))
        xbp = ctx.enter_context(tc.tile_pool(name="xbp", bufs=5))
        z0p = ctx.enter_context(tc.tile_pool(name="z0p", bufs=2))
        zqp = ctx.enter_context(tc.tile_pool(name="zqp", bufs=10))
        zgp = ctx.enter_context(tc.tile_pool(name="zgp", bufs=4))
        hrp = ctx.enter_context(tc.tile_pool(name="hrp", bufs=6))
        h3p = ctx.enter_context(tc.tile_pool(name="h3p", bufs=4))
        psp = ctx.enter_context(tc.tile_pool(name="psp", bufs=3, space="PSUM"))
        pssp = ctx.enter_context(tc.tile_pool(name="pssp", bufs=2, space="PSUM"))

        def load_z0(p):
            # z0 rides the ACT HWDGE queue: after startup that queue is
            # empty, so layer-0's stream never waits behind xb broadcasts
            # (and neither does the batched DMA-completion semaphore the
            # first layer-0 matmul waits on).
            z0t = z0p.tile([128, G0F, PW], bf16, name=f"z0t{p}", tag="z0")
            nc.scalar.dma_start(
                out=z0t,
                in_=z0_d[bass.ts(p, 128)].rearrange("k (g c) -> k g c", c=PW),
            )
            return z0t

        def load_xb(t, rh):
            # xb[p, ml, c] = x0[rh*16 + ml, t*512 + c] for all 128 p
            xb = xbp.tile([128, 16, NT], bf16, name=f"xb{t}_{rh}", tag="xb")
            nc.sync.dma_start(
                out=xb,
                in_=xc_d[2 * t + rh : 2 * t + rh + 1]
                .rearrange("o (m c) -> o m c", c=NT)
                .partition_broadcast(128),
            )
            return xb

        # ---- constants: small loads on the ACT HWDGE queue so they are
        # not serialized behind the first pair's big SP-queue streams ----
        ball = const.tile([O, 3], fp32)
        lws = const.tile([O, 3], bf16)
        nc.scalar.dma_start(out=ball[:, 0:1], in_=b0_d)
        nc.scalar.dma_start(out=ball[:, 1:2], in_=b1_d)
        nc.scalar.dma_start(out=ball[:, 2:3], in_=b2_d)
        nc.scalar.dma_start(out=lws, in_=lw_d)
        w0s = const.tile([128, G0F, O], bf16)
        w1s = const.tile([128, G, O], bf16)
        w2s = const.tile([128, G, O], bf16)
        nc.scalar.dma_start(out=w0s, in_=w0_d.rearrange("k (g o) -> k g o", o=O))
        z0_0 = load_z0(0)
        out_asm = const.tile([1, BL], fp32)
        pre = {0: (z0_0, None)}

        def evac_half(ps, li, p, half):
            # 4 copies of this half of h so DVE z-multiplies have
            # unit-stride (non-broadcast) operands; ACT is cheap.
            hr = hrp.tile([128, 4, NT], bf16, tag="hr", name=f"hr{li}_{p}_{half}")
            for j in range(4):
                nc.scalar.activation(hr[:, j], ps, Relu, bias=ball[:, li : li + 1])
            return hr

        def ps_half(tag, name):
            return psp.tile([128, NT], fp32, tag=tag, name=name)

        def emit_l0(p):
            z0t = pre[p][0]
            hrs = []
            for half in range(2):
                cs = bass.ts(half, NT)
                ps0 = ps_half(f"ps{half}", f"ps0_{p}_{half}")
                for g in range(G0F):
                    nc.tensor.matmul(
                        ps0, w0s[:, g], z0t[:, g, cs],
                        start=(g == 0), stop=(g == G0F - 1),
                    )
                hrs.append(evac_half(ps0, 0, p, half))
            return hrs  # [hrA, hrB]

        def emit_layer(p, li, hrA, hrB, wls, last):
            xb = pre[p][1]  # [colhalf*2 + rowhalf]
            hr_in = (hrA, hrB)

            def zfill(eng, zt, grp, half):
                # z[(m,h),c] for m in [4*grp, 4*grp+4), cols half*512+[0,512)
                ms = bass.ts(grp % 4, 4)
                eng.tensor_mul(zt, hr_in[half], xb[2 * half + grp // 4][:, ms])

            # GPSIMD half-B z for groups 2/5/7 (mid-layer deadlines).
            zgB = {}
            for grp in GP_GRPS:
                zgB[grp] = zgp.tile(
                    [128, 4, NT], bf16, tag="zg", name=f"zg{grp}_{li}_{p}"
                )
                zfill(nc.gpsimd, zgB[grp], grp, 1)

            outs = []
            for half in range(2):
                ps = ps_half(f"ps{half}", f"ps{li + 1}_{p}_{half}")
                for grp in range(8):
                    if half == 1 and grp in zgB:
                        zt = zgB[grp]
                    else:
                        zt = zqp.tile(
                            [128, 4, NT], bf16, tag="zq",
                            name=f"zq{grp}_{li}_{p}_{half}",
                        )
                        zfill(nc.vector, zt, grp, half)
                    for j in range(4):
                        k = 4 * grp + j
                        nc.tensor.matmul(
                            ps, wls[:, k], zt[:, j],
                            start=(k == 0), stop=(k == G - 1),
                        )
                if last:
                    h3 = h3p.tile([128, NT], bf16, tag="h3", name=f"h3_{p}_{half}")
                    nc.scalar.activation(h3, ps, Relu, bias=ball[:, li + 1 : li + 2])
                    outs.append(h3)
                else:
                    outs.append(evac_half(ps, li + 1, p, half))
            return outs

        def emit_score(p, hs1, hs2, hs3):
            for half in range(2):
                pss = pssp.tile([1, NT], fp32, tag="pss")
                nc.tensor.matmul(
                    pss, lws[:, 0:1], hs1[half][:, 0], start=True, stop=False
                )
                nc.tensor.matmul(
                    pss, lws[:, 1:2], hs2[half][:, 0], start=False, stop=False
                )
                nc.tensor.matmul(
                    pss, lws[:, 2:3], hs3[half], start=False, stop=True
                )
                bs = 16 * p + 8 * half
                nc.vector.tensor_reduce(
                    out=out_asm[0:1, bs : bs + 8],
                    in_=pss.rearrange("o (b d) -> o b d", d=D),
                    axis=AxX,
                    op=Add,
                )

        def load_pair(p):
            z0t = load_z0(p)
            xbs = [load_xb(2 * p, 0), load_xb(2 * p, 1), load_xb(2 * p + 1, 0),
                   load_xb(2 * p + 1, 1)]
            return (z0t, xbs)

        # Software-pipelined emission: pair p+1's DMA prefetch and
        # layer-0 are emitted between pair p's layer-1 and layer-2, and
        # pair p's scores are emitted one iteration LATE (mid pair p+1)
        # so the score matmuls + DVE reduces never head-of-line-block
        # the next pair's z fills on the in-order engine queues.
        # Pair 0's layer-0 is emitted BEFORE the first xb broadcasts so
        # the scheduler's batched DMA-completion wait on its first
        # matmul covers only z0+weights (~10us), not 23us of xb.
        h1s = {0: emit_l0(0)}
        xb_0 = [load_xb(0, 0), load_xb(0, 1), load_xb(1, 0), load_xb(1, 1)]
        nc.scalar.dma_start(out=w1s, in_=w1_d.rearrange("k (g o) -> k g o", o=O))
        nc.scalar.dma_start(out=w2s, in_=w2_d.rearrange("k (g o) -> k g o", o=O))
        pre[0] = (z0_0, xb_0)
        scoreq = {}
        for p in range(NP):
            hs1 = h1s.pop(p)
            if p - 1 in scoreq:
                emit_score(p - 1, *scoreq.pop(p - 1))
            hs2 = emit_layer(p, 0, hs1[0], hs1[1], w1s, last=False)
            if p + 1 < NP:
                pre[p + 1] = load_pair(p + 1)
                h1s[p + 1] = emit_l0(p + 1)
            hs3 = emit_layer(p, 1, hs2[0], hs2[1], w2s, last=True)
            scoreq[p] = (hs1, hs2, hs3)
            del pre[p]
        emit_score(NP - 1, *scoreq.pop(NP - 1))

        nc.scalar.dma_start(out=out_d, in_=out_asm)

    nc.compile()
    return nc


def prep_inputs(**inputs):
    """Host-side prep: per-core input maps (shard batch, permute weights)."""
    inp = np.asarray(inputs["input"], np.float32)
    W0 = np.asarray(inputs["W0"], np.float32)
    W1 = np.asarray(inputs["W1"], np.float32)
    W2 = np.asarray(inputs["W2"], np.float32)
    lw = np.asarray(inputs["lw"], np.float32)

    # Layers 1/2: WpT[(m*H+h), o] = Wr[o, h, m]; SBUF layout [k, (g, o)]
    # with chunk g == m (128 h-rows per chunk).
    def _prep_w(W, H):
        wp = W.reshape(O, H, M).transpose(2, 1, 0).reshape(H * M, O)
        g = H * M // 128
        return np.ascontiguousarray(
            wp.reshape(g, 128, O).transpose(1, 0, 2).reshape(128, g * O)
        ).astype(BF16)

    # Layer 0 folded: K index = upper-tri pair (a<=b); weight
    # W0f[o, (a,b)] = Wr0[o,a,b] + Wr0[o,b,a] (a<b), Wr0[o,a,a] (diag).
    ia, ib = _fold_pairs()
    Wr0 = W0.reshape(O, M, M)
    w0f = Wr0[:, ia, ib] + np.where(ia != ib, 1.0, 0.0)[None, :] * Wr0[:, ib, ia]
    w0f = np.concatenate(
        [w0f, np.zeros((O, K0F - w0f.shape[1]), np.float32)], axis=1
    )  # [O, 640]
    w0p = np.ascontiguousarray(
        w0f.T.reshape(G0F, 128, O).transpose(1, 0, 2).reshape(128, G0F * O)
    ).astype(BF16)

    w1p = _prep_w(W1, O)
    w2p = _prep_w(W2, O)
    b0 = np.asarray(inputs["b0"], np.float32).reshape(O, 1)
    b1 = np.asarray(inputs["b1"], np.float32).reshape(O, 1)
    b2 = np.asarray(inputs["b2"], np.float32).reshape(O, 1)
    lwseg = np.ascontiguousarray(lw.reshape(3, O).T).astype(BF16)

    shared = dict(w0p=w0p, w1p=w1p, w2p=w2p, b0=b0, b1=b1, b2=b2, lwseg=lwseg)
    in_maps = []
    for c in range(NCORES):
        xcore = np.ascontiguousarray(
            inp[BL * c : BL * (c + 1)].transpose(1, 0, 2).reshape(M, N)
        ).astype(BF16)
        # xc tile-major: row r = tile*2 + rowhalf -> 16 m-rows x 512 cols
        xc = np.ascontiguousarray(
            xcore.reshape(2, 16, 8, NT).transpose(2, 0, 1, 3).reshape(16, 16 * NT)
        )
        xf = xcore.astype(np.float32)
        z0f = (xf[ia] * xf[ib]).astype(BF16)  # [528, N]
        z0f = np.concatenate([z0f, np.zeros((K0F - z0f.shape[0], N), BF16)], axis=0)
        # z0 DMA layout: row (pair*128 + k), per row chunks g x 1024 cols
        z0 = np.ascontiguousarray(
            z0f.reshape(G0F, 128, NP, PW).transpose(2, 1, 0, 3).reshape(NP * 128, G0F * PW)
        )
        in_maps.append(dict(shared, xc=xc, z0=z0))
    return in_maps


def kernel(**inputs):
    import os

    from concourse import bass_utils

    if "nc" not in _CACHE:
        _CACHE["nc"] = _build()
    nc = _CACHE["nc"]

    in_maps = prep_inputs(**inputs)
    trace = os.environ.get("CIN_TRACE") == "1"
    res = bass_utils.run_bass_kernel_spmd(
        nc, in_maps, core_ids=list(range(NCORES)), trace=trace
    )
    _CACHE["last_res"] = res
    lb = float(np.asarray(inputs["lb"], np.float32).reshape(-1)[0])
    out = np.concatenate(
        [res.results[c]["out"].astype(np.float32).reshape(BL) for c in range(NCORES)]
    )
    return out + lb
